# revision 13
# baseline (speedup 1.0000x reference)
"""Sparse cosine-similarity attention kernel for Trainium2 (8 NeuronCores).

Problem: query [16,16,1,128], key [16,16,4096,128], mask [16,4096] int32
  scores[b,h,l] = <q,k_l> / max(|q||k_l|, 1e-8);  masked softmax over l.
Output: p_attn [16,16,4096] float32;  p_attn[b,h,l] = 0 where mask[b,l] = 0.

Sharding: batch dim split across 8 cores (2 batches/core, 32 (b,h) rows).

Sparsity: masked keys (~50%) never influence the output, so the host
compacts, per batch b, the kept key indices L_b = {l : mask[b,l]=1} (padded
to NK with index 0; padding killed later by padmask) and the kernel gathers
ONLY those K rows — roughly halving HBM traffic and compute vs dense.

Key layout trick: the host pre-casts K to bf16 and the SWDGE gather runs
with transpose=True, which lands each 4KB row (all 16 heads of a kept l)
as KT[d(part 0..127), h, i] directly in SBUF — the fully transposed layout
the PE matmuls need.  No PE transposes, no PSUM drains.

Per-core dataflow (compact position i, psum-bank blocks of <= 512):
  - dma_gather(transpose=True) per (b, block): KT slab [128(d), 16(h), sz]
  - squares: K2T = KT*KT elementwise on DVE (bf16 2x mode) per slab
  - dots:  psum[bh, i]  = sum_d MQ[d, bh] KT[d, h, i]   (masked-Q stationary,
    host-built bf16 [128, 32, 32]; accumulate matmuls over all 32 (b,h))
  - norms: psum[bh, i]  = sum_d MONES[d, bh] K2T[d, h, i]
  - tail per block: rk = exp(-0.5*ln(qn2*kn2)); e = exp(dots*rk)*padmask
  - softmax normalize in compact space, then gpsimd.ap_gather scatters the
    compact probabilities back to dense l-order (host-built inverse indices;
    trash slot at column 0 supplies the masked zeros), 512-col chunk stores.

softmax max-subtraction is dropped: scores are cosine similarities in [-1,1].
"""

import sys

if "/opt/trn_rl_repo" not in sys.path:
    sys.path.insert(0, "/opt/trn_rl_repo")

import numpy as np
import ml_dtypes

import concourse.bacc as bacc
import concourse.tile as tile
from concourse import mybir
from concourse.bass_utils import run_bass_kernel_spmd

F32 = mybir.dt.float32
BF16 = mybir.dt.bfloat16
I16 = mybir.dt.int16
AF = mybir.ActivationFunctionType
AX = mybir.AxisListType
NPBF16 = ml_dtypes.bfloat16

B, H, L, D = 16, 16, 4096, 128
NCORES = 8
BLOC = B // NCORES  # batches per core
NBH = BLOC * H  # 32 (b,h) rows per core
LB = 512  # max block size (psum bank = 512 fp32 per partition)

_ONE_SET = "natural_log_exp_and_others"  # contains Copy/Identity/Square/Ln/Exp


class _Bacc(bacc.Bacc):
    """Bacc that pins all activations to a single ACT table set, avoiding
    ~2.7us table reloads when Square and Ln/Exp interleave."""

    PIN_TABLES = True

    def insert_act_table_loads(self):
        super().insert_act_table_loads()
        if not self.PIN_TABLES:
            return
        from concourse.hw_specs import get_activation_tables

        names = list(get_activation_tables(self.m.arch).keys())
        target = names.index(_ONE_SET)
        first = True
        for fn in self.m.functions:
            for blk in fn.blocks:
                keep = []
                changed = False
                for inst in blk.instructions:
                    if type(inst).__name__ == "InstLoadActFuncSet":
                        if first:
                            inst.act_func_set_id = target
                            first = False
                            keep.append(inst)
                        else:
                            changed = True
                        continue
                    keep.append(inst)
                if changed:
                    del blk.instructions[:]
                    for i in keep:
                        blk.instructions.append(i)


def _blocks(nk):
    """Split nk compact columns into PSUM-bank blocks of <= LB columns.
    Any short remainder block goes FIRST: its smaller gather fills the
    pipeline sooner, so the first matmuls start earlier."""
    rem = nk % LB
    out = []
    c = 0
    if rem:
        out.append((0, rem))
        c = rem
    while c < nk:
        out.append((c, LB))
        c += LB
    return out


def build_module(nk, variant="full", reps=1):
    assert nk % 128 == 0
    blocks = _blocks(nk)
    nblk = len(blocks)
    nc = _Bacc(
        "TRN2",
        target_bir_lowering=False,
        debug=False,
        num_devices=NCORES,
        num_swdge_queues=1,
        dynamic_dma_scratch_size=32768,  # 2048-desc ring: 4 gathers in flight
    )
    q_d = nc.dram_tensor("query", [BLOC, H, 1, D], F32, kind="ExternalInput").ap()
    k_d = nc.dram_tensor("key", [BLOC, L, H, D], BF16, kind="ExternalInput").ap()
    idx_d = nc.dram_tensor(
        "kidx", [BLOC, 128, nk // 16], I16, kind="ExternalInput"
    ).ap()
    inv_d = nc.dram_tensor("kinv", [NBH, L // 16], I16, kind="ExternalInput").ap()
    pm_d = nc.dram_tensor("padmask", [NBH, nk], F32, kind="ExternalInput").ap()
    mq_d = nc.dram_tensor("mq", [128, NBH, NBH], BF16, kind="ExternalInput").ap()
    mo_d = nc.dram_tensor("mones", [128, NBH, NBH], BF16, kind="ExternalInput").ap()
    o_d = nc.dram_tensor("out", [BLOC, H, L], F32, kind="ExternalOutput").ap()

    with tile.TileContext(nc) as tc:
        with (
            tc.tile_pool(name="persist", bufs=1) as pers,
            tc.tile_pool(name="ktp", bufs=4) as ktp,
            tc.tile_pool(name="k2p", bufs=3) as k2p,
            tc.tile_pool(name="psd", bufs=2, space="PSUM") as psd,
            tc.tile_pool(name="psn", bufs=2, space="PSUM") as psn,
        ):
            # ------------- prologue: gather deps first ------------
            idxs = []
            for b in range(BLOC):
                idx_b = pers.tile([128, nk // 16], I16, tag=f"idx{b}", name=f"idx{b}")
                nc.sync.dma_start(idx_b[:], idx_d[b])
                idxs.append(idx_b)

            qsb = pers.tile([NBH, D], F32, tag="qsb")
            nc.scalar.dma_start(qsb[:], q_d.rearrange("b h o d -> (b h) (o d)"))

            # host-built masked stationaries (bf16):
            # MQ[:, bh, :] has q_bh in column bh, zeros elsewhere.
            # MONES[:, bh, :] has ones in column bh.
            mq = pers.tile([128, NBH, NBH], BF16, tag="mq")
            nc.sync.dma_start(mq[:], mq_d)
            mones = pers.tile([128, NBH, NBH], BF16, tag="mones")
            nc.scalar.dma_start(mones[:], mo_d)

            inv_sb = pers.tile([NBH, L // 16], I16, tag="inv")
            nc.sync.dma_start(inv_sb[:], inv_d)
            pmask = pers.tile([NBH, nk], F32, tag="pmask")
            nc.scalar.dma_start(pmask[:], pm_d)

            # qn2[bh] = |q_bh|^2  (fused square+reduce on DVE)
            junkq = pers.tile([NBH, D], F32, tag="junkq")
            qn2 = pers.tile([NBH, 1], F32, tag="qn2")
            nc.vector.scalar_tensor_tensor(
                out=junkq[:],
                in0=qsb[:],
                scalar=1.0,
                in1=qsb[:],
                op0=mybir.AluOpType.mult,
                op1=mybir.AluOpType.mult,
                accum_out=qn2[:],
            )

            # compact-space scores; column 0 is the ap_gather trash slot
            # (inv indices are shifted by +1; masked l points at column 0)
            scomp = pers.tile([NBH, nk + 1], F32, tag="scomp")
            nc.vector.memset(scomp[:, 0:1], 0.0)
            kn2d = pers.tile([NBH, nk], F32, tag="kn2d")
            partials = pers.tile([NBH, nblk], F32, tag="partials")
            dense = pers.tile([NBH, L], F32, tag="dense")

            # ---------------- main loop -----------------
            def one_pass():
                def dense_chunks(j0, j1):
                    # dense l-chunks [j0, j1) in one gather; chunk j only
                    # reads compact columns <= j*LB+LB (inv[l] <= l), so the
                    # input AP can stop there (kills WAR deps on scomp).
                    sl = slice(j0 * LB, j1 * LB)
                    if variant == "noapg":
                        nc.vector.memset(dense[:, sl], 0.0)
                        return
                    w = min(j1 * LB, nk) + 1
                    nc.gpsimd.ap_gather(
                        dense[:, sl].unsqueeze(2),
                        scomp[:, 0:w].unsqueeze(2),
                        inv_sb[:, j0 * (LB // 16) : j1 * (LB // 16)],
                        NBH,
                        w,
                        1,
                        (j1 - j0) * LB,
                    )

                def epilogue(j):
                    # op order matters: the kn2 chain (needs pbn only) is
                    # emitted first on both engine streams so it can run
                    # while the dots matmuls of this block still execute.
                    c0, sz = blocks[j]
                    sl = slice(c0, c0 + sz)
                    sl1 = slice(c0 + 1, c0 + sz + 1)
                    nc.scalar.copy(kn2d[:, sl], pbn[:, 0:sz])
                    nc.vector.tensor_scalar_mul(kn2d[:, sl], kn2d[:, sl], qn2[:])
                    nc.scalar.activation(kn2d[:, sl], kn2d[:, sl], AF.Ln)
                    nc.scalar.activation(
                        kn2d[:, sl], kn2d[:, sl], AF.Exp, scale=-0.5
                    )
                    nc.vector.tensor_copy(scomp[:, sl1], pbd[:, 0:sz])
                    nc.vector.tensor_mul(scomp[:, sl1], scomp[:, sl1], kn2d[:, sl])
                    nc.scalar.activation(scomp[:, sl1], scomp[:, sl1], AF.Exp)
                    # e *= padmask, with per-row partial sums (one DVE op)
                    nc.vector.scalar_tensor_tensor(
                        out=scomp[:, sl1],
                        in0=scomp[:, sl1],
                        scalar=1.0,
                        in1=pmask[:, sl],
                        op0=mybir.AluOpType.mult,
                        op1=mybir.AluOpType.mult,
                        accum_out=partials[:, j : j + 1],
                    )

                for j, (c0, sz) in enumerate(blocks):
                    kts, k2ts = [], []
                    for b in range(BLOC):
                        kt = ktp.tile([128, H, sz], BF16, tag="kt", name="kt")
                        nc.gpsimd.dma_gather(
                            kt[:],
                            k_d[b].rearrange("l h d -> l (h d)"),
                            idxs[b][:, c0 // 16 : (c0 + sz) // 16],
                            sz,
                            sz,
                            H * D,
                            transpose=True,
                            queue_num=0,
                            single_packet=False,
                        )
                        kts.append(kt)
                        if variant == "gatheronly":
                            continue
                        k2 = k2p.tile([128, H, sz], BF16, tag="k2", name="k2")
                        nc.vector.tensor_mul(k2[:], kt[:], kt[:])
                        k2ts.append(k2)
                    if variant == "gatheronly":
                        if j == nblk - 1:
                            nc.vector.memset(dense[:], 0.0)
                            nc.vector.tensor_add(
                                dense[:, 0:D],
                                dense[:, 0:D],
                                kts[0][0:NBH, 0, 0:D],
                            )
                        continue
                    pbd = psd.tile([NBH, LB], F32, tag="d", name="pbd")
                    pbn = psn.tile([NBH, LB], F32, tag="n", name="pbn")
                    if variant != "nomm":
                        # norms first: frees the kn2 epilogue chain to
                        # overlap with this block's dots matmuls
                        for b in range(BLOC):
                            for h in range(H):
                                bh = b * H + h
                                nc.tensor.matmul(
                                    pbn[:, 0:sz],
                                    mones[:, bh, :],
                                    k2ts[b][:, h, :],
                                    start=(bh == 0),
                                    stop=(bh == NBH - 1),
                                    skip_group_check=True,
                                )
                        for b in range(BLOC):
                            for h in range(H):
                                bh = b * H + h
                                nc.tensor.matmul(
                                    pbd[:, 0:sz],
                                    mq[:, bh, :],
                                    kts[b][:, h, :],
                                    start=(bh == 0),
                                    stop=(bh == NBH - 1),
                                    skip_group_check=True,
                                )
                    else:
                        nc.vector.memset(pbd[:], 0.0)
                        nc.vector.memset(pbn[:], 1.0)
                    epilogue(j)

                if variant == "gatheronly":
                    nc.sync.dma_start(
                        o_d.rearrange("b h l -> (b h) l"), dense[:]
                    )
                    return

                # ---------------- normalize + dense scatter ----------
                # all dense scatters live in the tail so the Pool engine's
                # instruction stream never blocks gather generation mid-loop
                tot = pers.tile([NBH, 1], F32, tag="tot", name="tot")
                nc.vector.reduce_sum(tot[:], partials[:], axis=AX.X)
                srec = pers.tile([NBH, 1], F32, tag="srec", name="srec")
                nc.vector.reciprocal(srec[:], tot[:])

                nchk = L // LB
                for j0 in range(0, nchk, nchk // 2):
                    j1 = j0 + nchk // 2
                    dense_chunks(j0, j1)
                    sl = slice(j0 * LB, j1 * LB)
                    nc.vector.tensor_scalar_mul(dense[:, sl], dense[:, sl], srec[:])
                    nc.sync.dma_start(
                        o_d.rearrange("b h l -> (b h) l")[:, sl], dense[:, sl]
                    )

            if reps == 1:
                one_pass()
            else:
                with tc.For_i(0, reps, 1):
                    one_pass()

    nc.compile()
    return nc


_CACHE = {}


def _get_module(nk, variant="full"):
    key = (nk, variant)
    if key not in _CACHE:
        _CACHE[key] = build_module(nk, variant)
    return _CACHE[key]


def _round_up(x, m):
    return (x + m - 1) // m * m


def _pick_nk(mask):
    counts = (np.asarray(mask) != 0).sum(axis=1)
    return max(_round_up(int(counts.max()), 128), 512)


def _make_in_maps(query, key, mask, nk):
    query = np.asarray(query, np.float32)
    key_bf = np.ascontiguousarray(
        np.asarray(key).astype(NPBF16).transpose(0, 2, 1, 3)
    )  # [B, L, H, D] bf16
    mask = np.asarray(mask)
    mones = np.zeros((128, NBH, NBH), NPBF16)
    for bh in range(NBH):
        mones[:, bh, bh] = 1.0
    in_maps = []
    for c in range(NCORES):
        b0 = c * BLOC
        idx = np.zeros((BLOC, 128, nk // 16), np.int16)
        inv = np.zeros((NBH, L // 16), np.int16)
        pm = np.zeros((NBH, nk), np.float32)
        mq = np.zeros((128, NBH, NBH), NPBF16)
        for bl in range(BLOC):
            kept = np.flatnonzero(mask[b0 + bl]).astype(np.int64)
            nb = len(kept)
            assert nb <= nk, f"kept count {nb} exceeds NK {nk}"
            flat = np.zeros(nk, np.int16)  # pad with row 0 (full-valid)
            flat[:nb] = kept.astype(np.int16)
            wrapped = flat.reshape(nk // 16, 16).T  # [16, nk/16]
            idx[bl] = np.tile(wrapped, (8, 1))
            invf = np.zeros(L, np.int16)  # trash slot at column 0
            invf[kept] = np.arange(1, nb + 1, dtype=np.int16)
            inv[bl * 16 : (bl + 1) * 16] = invf.reshape(L // 16, 16).T
            pm[bl * H : (bl + 1) * H, :nb] = 1.0
        q = query[b0 : b0 + BLOC].reshape(NBH, D)
        for bh in range(NBH):
            mq[:, bh, bh] = q[bh].astype(NPBF16)
        in_maps.append(
            {
                "query": np.ascontiguousarray(query[b0 : b0 + BLOC]),
                "key": np.ascontiguousarray(key_bf[b0 : b0 + BLOC]),
                "kidx": idx,
                "kinv": inv,
                "padmask": pm,
                "mq": mq,
                "mones": mones,
            }
        )
    return in_maps


def _run(query, key, mask, trace=False, nk=None):
    if nk is None:
        nk = _pick_nk(mask)
    nc = _get_module(nk)
    in_maps = _make_in_maps(query, key, mask, nk)
    res = run_bass_kernel_spmd(
        nc, in_maps, core_ids=list(range(NCORES)), trace=trace
    )
    out = np.concatenate([r["out"] for r in res.results], axis=0)
    return out, res


def kernel(query, key, mask):
    out, _ = _run(np.asarray(query), np.asarray(key), np.asarray(mask))
    return out
